# revision 54
# baseline (speedup 1.0000x reference)
"""Multi-head attention (LoRA QKV + ALiBi + causal softmax + output proj) on 8 TRN2 cores.

Sharding: core = (batch b in 0..3, head-set in {A, B}); each core handles one batch
element and 8 of the 16 heads, chosen so per-core work balances (see below).

Key algorithmic fact: the reference ALiBi bias is +slope*(i-j) with causal j<=i, so
softmax weight concentrates on the EARLIEST keys (small j).  For head h the keys
beyond j_cut = 40/slope_h contribute relative weight < e^-21 -- dropping them is
exact at fp32.  Per-head j-tile counts (128-wide):
  jn = [1,1,1,2,2,3,4,5,8,10,15,16,16,16,16,16]
Heads are paired into 128-partition strips; the SPMD program uses slot j-depths
SLOT_JN = [16,16,5,2] (the elementwise union of both sets' sorted pair depths); a
head whose own jn is smaller than its slot simply has gv = 0 on the extra tiles
(zero contribution to numerator and denominator -- exact).

  set A pairs: (12,13) (14,15) (4,5) (0,1)   jns 16,16,3,1
  set B pairs: (10,11) (8,9)  (6,7) (2,3)   jns 16,10,5,2

LoRA is folded into effective weights on the host (W_eff = W + 2*A@B, exact).
Numerics: x, Wq/Wk/Wv, q, k, p=exp(s), v' in bf16 (measured end-to-end error
~4e-3 vs the 2e-2 gate); all psum accumulation fp32; output projection fp32.

On-core math per slot (pair of heads), analytic softmax (no max pass):
  qT[d,t], kT[d,j]: bf16 projections, wq pre-scaled by 1/sqrt(dh)
  s[j,i]  = sum_d kT[d,j] qT[d,i]      (psum fp32, two 64-contraction matmuls)
  p[j,i]  = exp(s)  (bf16)             (ALiBi j-part rides on v' as gv[j] =
                                        exp(-slope*j - C); the i-part cancels)
  causal: affine_select zeroes j>i on the 128-wide diagonal block
  v'[j,:] = [v_h0*gv0 | gv0 | v_h1*gv1 | gv1]   (130 cols, bf16)
  pv[i, itl*65+d] = sum_j p[j,i] v'[j,d]   (TRANSPOSED accumulation: p is the
                    stationary operand, free dim = 65, so PV costs 65 rows per
                    (i-tile, j-tile, head) instead of 128; denominator lands at
                    col 64 of each 65-block = a per-PARTITION scalar)
  o[t, d] = pv[t, d] * recip(denominator)  (DVE tensor_scalar per i-tile/head;
                    no broadcast matmul, no PSUM staging, no partition shift)
  oT = transpose(o)  (PE transpose per [128,128] tile, then one copy per chunk)
  out[t,e] = sum_d oT[d,t] wpT[d,e]    (fp32; host adds the two cores per batch)

Schedule: x streams in 4 token-chunks of 512. P-work (q for all slots, k/v only
up to each slot's depth, v packed across active slots for wide matmuls) and the
output projection are split into ~2-matmul units and fed through filler queues,
popped between attention steps so the PE always has independent work covering
the exp latency (pv lags its score matmul by two steps; lagged normalize +
transpose units pop with priority).  P(x-chunk w+2) is prefetched during window
w; projections are held back for the filler-starved final window.  PSUM note:
start=True zeroes the whole bank, so only the first pv write of a chunk sets
it; the other i-tile accumulators rely on that zeroing (skip_group_check).
"""

import math
from contextlib import ExitStack

import numpy as np
import ml_dtypes

import concourse.bacc as bacc
import concourse.mybir as mybir
import concourse.tile as tile
from concourse.bass_utils import run_bass_kernel_spmd

T, E, DH, H = 2048, 1024, 64, 16
NKT = 8             # contraction tiles of 128 over E
CB = 12.0           # safety constant in the analytic softmax max
SLOT_JN = [16, 16, 5, 2]          # j-tile depth per slot (shared SPMD structure)
SLOT_BASE = [0, 16, 32, 37]       # cumsum of SLOT_JN
NJT_TOT = 39                      # sum(SLOT_JN)
JN_HEAD = [1, 1, 1, 2, 2, 3, 4, 5, 8, 10, 15, 16, 16, 16, 16, 16]
SET_A = [(12, 13), (14, 15), (4, 5), (0, 1)]
SET_B = [(10, 11), (8, 9), (6, 7), (2, 3)]

BF16 = ml_dtypes.bfloat16

_NC_CACHE = None


def _build_nc():
    f32 = mybir.dt.float32
    f32r = mybir.dt.float32r
    bf16 = mybir.dt.bfloat16
    Exp = mybir.ActivationFunctionType.Exp

    nc = bacc.Bacc(trn_type="TRN2", target_bir_lowering=False, debug=False)
    xT_d = nc.declare_dram_parameter("xT", [E, T], bf16, isOutput=False)
    # wqT/wkT are stored k-major on the host: [128, slot, ktile, 128]
    wqT_d = nc.declare_dram_parameter("wqT", [128, 4096], bf16, isOutput=False)
    wkT_d = nc.declare_dram_parameter("wkT", [128, 4096], bf16, isOutput=False)
    wvT_d = nc.declare_dram_parameter("wvT", [E, 512], bf16, isOutput=False)
    wpT_d = nc.declare_dram_parameter("wpT", [512, E], f32, isOutput=False)
    gv_d = nc.declare_dram_parameter("gvt", [128, 128], f32, isOutput=False)
    id_d = nc.declare_dram_parameter("identb", [128, 128], bf16, isOutput=False)
    out_d = nc.declare_dram_parameter("out", [T, E], f32, isOutput=True)

    with ExitStack() as st:
        tc = st.enter_context(tile.TileContext(nc))
        ps = st.enter_context(tc.tile_pool(name="ps", bufs=1, space="PSUM"))
        # psum budget: acc(2) + s(2x2) + pv(2) = 8 banks
        sb_r = st.enter_context(tc.tile_pool(name="sbr", bufs=1, side="right"))
        sb_x = st.enter_context(tc.tile_pool(name="sbx", bufs=1, side="left"))
        sb_l = st.enter_context(tc.tile_pool(name="sbl", bufs=1, side="left"))

        # ---------- persistent SBUF tiles ----------
        # x chunk tiles: 8 contraction tiles x [128, 512] per chunk, double
        # buffered by re-allocating from the pool each chunk
        xts_by_chunk = {}
        qts = [sb_l.tile([128, T], bf16, tag=f"qt{s}", bufs=1, name=f"qt{s}")
               for s in range(4)]
        kts = [sb_l.tile([128, SLOT_JN[s] * 128], bf16, tag=f"kt{s}", bufs=1,
                         name=f"kt{s}") for s in range(4)]
        vts = [[sb_r.tile([128, 130], bf16, tag=f"v{s}_{j}", bufs=1,
                          name=f"v{s}_{j}") for j in range(SLOT_JN[s])]
               for s in range(4)]
        oTs = [sb_r.tile([128, T], f32r, tag=f"ot{s}", bufs=1, name=f"ot{s}")
               for s in range(4)]
        wq = [None] * 4
        wk = [None] * 4
        wvs = [None] * NKT
        wps = [None] * 8
        gv_sb = sb_r.tile([128, 128], f32, tag="gv", bufs=1)
        ident_t = sb_r.tile([128, 128], bf16, tag="idt", bufs=1)
        # o in [t, d] layout before the pre-projection transpose
        oT_t = [sb_r.tile([128, 512], bf16, tag=f"otr{tt}", bufs=1,
                          name=f"otr{tt}") for tt in range(16)]
        scr = sb_r.tile([1, 2], f32, tag="scr", bufs=1)
        scrb = sb_r.tile([1, 2], bf16, tag="scrb", bufs=1)

        # ---------- DMA emitters ----------
        def dma_x_chunk(ck):
            tiles = [sb_x.tile([128, 512], bf16, tag=f"xt{k}", bufs=2,
                               name=f"xt{k}_c{ck}") for k in range(NKT)]
            for k in range(NKT):
                nc.sync.dma_start(
                    out=tiles[k][:],
                    in_=xT_d[k * 128:(k + 1) * 128, ck * 512:(ck + 1) * 512])
            xts_by_chunk[ck] = tiles

        def dma_wqk(s):
            for which, wd, dst in (("q", wqT_d, wq), ("k", wkT_d, wk)):
                wt = sb_l.tile([128, 1024], bf16, tag=f"w{which}{s}", bufs=1,
                               name=f"w{which}{s}")
                if s == 0 and which == "q":
                    # split the startup-critical first weight tile across two
                    # queues so the first q matmul can issue sooner
                    nc.gpsimd.dma_start(out=wt[:, 0:512],
                                        in_=wd[:, 0:512])
                    nc.scalar.dma_start(out=wt[:, 512:1024],
                                        in_=wd[:, 512:1024])
                else:
                    nc.gpsimd.dma_start(out=wt[:],
                                        in_=wd[:, s * 1024:(s + 1) * 1024])
                dst[s] = wt

        def dma_wv():
            for k in range(NKT):
                t = sb_l.tile([128, 512], bf16, tag=f"wv{k}", bufs=1, name=f"wv{k}")
                nc.gpsimd.dma_start(out=t[:], in_=wvT_d[k * 128:(k + 1) * 128, :])
                wvs[k] = t

        def dma_wp():
            for i in range(8):  # i = s*2 + ec
                s, ec = i // 2, i % 2
                t = sb_l.tile([128, 512], f32r, tag=f"wp{i}", bufs=1, name=f"wp{i}")
                nc.gpsimd.dma_start(
                    out=t[:],
                    in_=wpT_d[s * 128:(s + 1) * 128,
                              ec * 512:(ec + 1) * 512].bitcast(f32r))
                wps[i] = t

        # ---------- phase P emitters (unit-queue variants) ----------
        # Each P/proj group is split into "units" of ~2 matmuls; units are
        # popped one per attention jt-step so the PE always has independent
        # work between a score matmul and its dependent pv matmul (covering
        # the exp latency), with whole groups still accumulating into one
        # psum tile (interleaved matmuls to other banks are fine).

        def units_q(s, tck):
            xts = xts_by_chunk[tck]
            state = {}

            def u(i):
                def f():
                    if i == 0:
                        state["pq"] = ps.tile([128, 512], f32, tag="acc", bufs=2, name="pacc")
                    pq = state["pq"]
                    for k in (2 * i, 2 * i + 1):
                        nc.tensor.matmul(pq[:], wq[s][:, k * 128:(k + 1) * 128],
                                         xts[k][:],
                                         start=(k == 0), stop=(k == NKT - 1))
                    if i == 3:
                        nc.vector.tensor_copy(
                            qts[s][:, tck * 512:(tck + 1) * 512], pq[:])
                return f
            return [(427, u(i)) for i in range(4)]

        def units_k(s, tck):
            width = min(SLOT_JN[s] * 128 - tck * 512, 512)
            if width <= 0:
                return []
            xts = xts_by_chunk[tck]
            state = {}

            def u(i):
                def f():
                    if i == 0:
                        state["pq"] = ps.tile([128, 512], f32, tag="acc", bufs=2, name="pacc")
                    pq = state["pq"]
                    for k in (2 * i, 2 * i + 1):
                        nc.tensor.matmul(pq[:, 0:width],
                                         wk[s][:, k * 128:(k + 1) * 128],
                                         xts[k][:, 0:width],
                                         start=(k == 0), stop=(k == NKT - 1))
                    if i == 3:
                        nc.vector.tensor_copy(
                            kts[s][:, tck * 512:tck * 512 + width],
                            pq[:, 0:width])
                return f
            return [(int(width * 2 * 0.417), u(i)) for i in range(4)]

        def units_v(tt):
            nact = sum(1 for s in range(4) if tt < SLOT_JN[s])
            if nact == 0:
                return []
            tl = tt % 4
            xts = xts_by_chunk[tt // 4]
            state = {}

            def u(i):
                def f():
                    if i == 0:
                        state["pvm"] = ps.tile([128, 512], f32, tag="acc", bufs=2, name="pacc")
                    pvm = state["pvm"]
                    for k in (2 * i, 2 * i + 1):
                        nc.tensor.matmul(pvm[:, 0:128 * nact],
                                         xts[k][:, tl * 128:(tl + 1) * 128],
                                         wvs[k][:, 0:128 * nact],
                                         start=(k == 0), stop=(k == NKT - 1))
                    if i == 3:
                        for s in range(nact):
                            vt = vts[s][tt]
                            g0 = 2 * (SLOT_BASE[s] + tt)
                            nc.vector.tensor_scalar_mul(
                                vt[:, 0:64], pvm[:, s * 128:s * 128 + 64],
                                gv_sb[:, g0:g0 + 1])
                            nc.vector.tensor_scalar_mul(
                                vt[:, 65:129],
                                pvm[:, s * 128 + 64:s * 128 + 128],
                                gv_sb[:, g0 + 1:g0 + 2])
                            v3 = vt.rearrange("p (h c) -> p h c", c=65)
                            nc.vector.tensor_copy(
                                v3[:, :, 64:65],
                                gv_sb[:, g0:g0 + 2].rearrange(
                                    "p (h c) -> p h c", c=1))
                return f
            return [(int(nact * 128 * 2 * 0.417), u(i)) for i in range(4)]

        def units_proj(tt, ec):
            state = {}

            def u(i):
                def f():
                    if i == 0:
                        state["po"] = ps.tile([128, 512], f32, tag="acc", bufs=2, name="pacc")
                    po = state["po"]
                    for s in (2 * i, 2 * i + 1):
                        nc.tensor.matmul(po[:], oTs[s][:, tt * 128:(tt + 1) * 128],
                                         wps[s * 2 + ec][:],
                                         start=(s == 0), stop=(s == 3))
                    if i == 1:
                        ob = sb_l.tile([128, 512], f32, tag="ob", bufs=4)
                        if tt >= 14 and ec == 1:
                            nc.scalar.activation(
                                ob[:], po[:], mybir.ActivationFunctionType.Copy)
                        else:
                            nc.vector.tensor_copy(ob[:], po[:])
                        nc.sync.dma_start(
                            out=out_d[tt * 128:(tt + 1) * 128,
                                      ec * 512:(ec + 1) * 512],
                            in_=ob[:])
                return f
            return [(427, u(i)) for i in range(2)]

        fill_q = []   # P-projection units: must drain before the next x-chunk
        proj_q = []   # output-projection units: only ordering vs. final tail
        norm_q = []   # lagged normalize work; popped with priority

        def pop_filler(min_ns=600):
            got = 0
            while norm_q and got < min_ns:
                cost, fn = norm_q.pop(0)
                fn()
                got += cost
            while fill_q and got < min_ns:
                cost, fn, _ = fill_q.pop(0)
                fn()
                got += cost
            while proj_q and got < min_ns:
                cost, fn = proj_q.pop(0)
                fn()
                got += cost

        def flush_fillers(upto_win=None, proj=False):
            while norm_q:
                norm_q.pop(0)[1]()
            while fill_q and (upto_win is None or fill_q[0][2] <= upto_win):
                fill_q.pop(0)[1]()
            while proj and proj_q:
                proj_q.pop(0)[1]()

        # ---------- attention emitter: lag-1 pv + one filler per jt ----------
        # i-window width AW=256 keeps each s psum tile to one bank, freeing
        # banks for pv bufs=4 (no write-after-read wait on chunk turnover)
        AW = 512
        NAC = T // AW          # number of attention i-chunks
        ACT_PER = AW // 128    # j-tiles entering the causal window per chunk

        def emit_attn(s, c):
            njt = min(SLOT_JN[s], ACT_PER * c + ACT_PER)
            # transposed accumulation: pvh[h][:, itl*65 : +65] holds
            # out[i, d] for i-tile itl of this window (denominator at col 64)
            pvh = [ps.tile([128, 512], f32, tag="pv", bufs=2, name="pvh")
                   for _ in range(2)]
            stash = [None] * njt

            def emit_pv(jt):
                p01, cw = stash[jt]
                r = jt - ACT_PER * c
                first_itl = max(r, 0)
                for itl in range(first_itl, 4):
                    pcol = itl * 128 - (AW - cw)
                    stop = (jt == min(njt - 1, ACT_PER * c + itl))
                    for h in range(2):
                        # start=True zeroes the whole psum bank, so only the
                        # first write of the chunk carries it; later i-tiles
                        # accumulate onto the already-zeroed bank
                        nc.tensor.matmul(
                            pvh[h][:, itl * 65:itl * 65 + 65],
                            p01[:, h * AW + pcol:h * AW + pcol + 128],
                            vts[s][jt][:, h * 65:h * 65 + 65],
                            start=(jt == 0 and itl == first_itl), stop=stop,
                            skip_group_check=True)

            for jt in range(njt):
                r = jt - ACT_PER * c
                cw = AW - 128 * r if r > 0 else AW
                ioff = c * AW + (AW - cw)
                s01 = ps.tile([128, 2 * AW], f32, tag="s", bufs=2)
                nc.tensor.matmul(s01[:, 0:cw],
                                 kts[s][0:64, jt * 128:(jt + 1) * 128],
                                 qts[s][0:64, ioff:ioff + cw],
                                 start=True, stop=True)
                nc.tensor.matmul(s01[:, AW:AW + cw],
                                 kts[s][64:128, jt * 128:(jt + 1) * 128],
                                 qts[s][64:128, ioff:ioff + cw],
                                 start=True, stop=True)
                p01 = sb_l.tile([128, 2 * AW], bf16, tag="pt", bufs=3)
                s3 = s01.rearrange("p (h m) -> p h m", h=2)
                p3 = p01.rearrange("p (h m) -> p h m", h=2)
                nc.scalar.activation(p3[:, :, 0:cw], s3[:, :, 0:cw], Exp)
                if r >= 0:
                    for off in (0, AW):
                        nc.gpsimd.affine_select(
                            out=p01[:, off:off + 128], in_=p01[:, off:off + 128],
                            compare_op=mybir.AluOpType.is_ge, fill=0.0,
                            base=0, pattern=[[1, 128]], channel_multiplier=-1)
                stash[jt] = (p01, cw)
                pop_filler()
                if jt > 1:
                    emit_pv(jt - 2)
            if njt > 1:
                pop_filler()
                emit_pv(njt - 2)
            pop_filler()
            emit_pv(njt - 1)
            # normalize: denominators are per-PARTITION scalars (col 64 of each
            # 65-block), so 1/Z broadcasts along the free dim on the DVE with
            # no ones-matmul, PSUM staging, or partition-shift DMA
            rr = sb_l.tile([128, 8], f32r, tag="rr", bufs=2, name="rr")
            r4 = rr.rearrange("p (h i c) -> p h i c", h=2, c=1)
            with nc.allow_low_precision("f32r reciprocal of softmax denom"):
                for h in range(2):
                    pv4 = pvh[h][:, 0:260].rearrange("p (i c) -> p i c", c=65)
                    nc.vector.reciprocal(r4[:, h], pv4[:, :, 64:65])

            def norm_tail(pvh=pvh, rr=rr):
                for itl in range(4):
                    tt = ACT_PER * c + itl
                    for h in range(2):
                        nc.vector.tensor_scalar_mul(
                            oT_t[tt][:, s * 128 + h * 64:s * 128 + h * 64 + 64],
                            pvh[h][:, itl * 65:itl * 65 + 64],
                            rr[:, h * 4 + itl:h * 4 + itl + 1])
                # PE transpose [t, d] -> [d, t] for the output projection
                tp = ps.tile([128, 512], bf16, tag="acc", bufs=2, name="tp")
                for itl in range(4):
                    nc.tensor.transpose(tp[:, itl * 128:(itl + 1) * 128],
                                        oT_t[ACT_PER * c + itl][:, s * 128:(s + 1) * 128],
                                        ident_t[:])
                nc.vector.tensor_copy(oTs[s][:, c * AW:(c + 1) * AW], tp[:])
            if c == NAC - 1:
                norm_tail()
            else:
                norm_q.append((400, norm_tail))

        def units_proj(tt, ec):
            state = {}

            def u(i):
                def f():
                    if i == 0:
                        state["po"] = ps.tile([128, 512], f32, tag="acc", bufs=2, name="pacc")
                    po = state["po"]
                    for s in (2 * i, 2 * i + 1):
                        nc.tensor.matmul(po[:], oTs[s][:, tt * 128:(tt + 1) * 128],
                                         wps[s * 2 + ec][:],
                                         start=(s == 0), stop=(s == 3))
                    if i == 1:
                        ob = sb_l.tile([128, 512], f32, tag="ob", bufs=4)
                        if tt >= 14 and ec == 1:
                            nc.scalar.activation(
                                ob[:], po[:], mybir.ActivationFunctionType.Copy)
                        else:
                            nc.vector.tensor_copy(ob[:], po[:])
                        nc.sync.dma_start(
                            out=out_d[tt * 128:(tt + 1) * 128,
                                      ec * 512:(ec + 1) * 512],
                            in_=ob[:])
                return f
            return [(427, u(i)) for i in range(2)]

        fill_q = []   # P-projection units: must drain before the next x-chunk
        proj_q = []   # output-projection units: only ordering vs. final tail
        norm_q = []   # lagged normalize work; popped with priority

        def pop_filler(min_ns=600):
            got = 0
            while norm_q and got < min_ns:
                cost, fn = norm_q.pop(0)
                fn()
                got += cost
            while fill_q and got < min_ns:
                cost, fn, _ = fill_q.pop(0)
                fn()
                got += cost
            while proj_q and got < min_ns:
                cost, fn = proj_q.pop(0)
                fn()
                got += cost

        def flush_fillers(upto_win=None, proj=False):
            while norm_q:
                norm_q.pop(0)[1]()
            while fill_q and (upto_win is None or fill_q[0][2] <= upto_win):
                fill_q.pop(0)[1]()
            while proj and proj_q:
                proj_q.pop(0)[1]()

        # ---------- attention emitter: lag-1 pv + one filler per jt ----------
        # i-window width AW=256 keeps each s psum tile to one bank, freeing
        # banks for pv bufs=4 (no write-after-read wait on chunk turnover)
        AW = 512
        NAC = T // AW          # number of attention i-chunks
        ACT_PER = AW // 128    # j-tiles entering the causal window per chunk

        def emit_attn(s, c):
            njt = min(SLOT_JN[s], ACT_PER * c + ACT_PER)
            pv0 = ps.tile([128, AW], f32, tag="pv", bufs=2)
            pv1 = ps.tile([128, AW], f32, tag="pv", bufs=2)
            stash = [None] * njt  # (p01, cw) per jt, pv emitted one step late

            def emit_pv(jt):
                p01, cw = stash[jt]
                nc.tensor.matmul(pv0[0:65, AW - cw:AW], vts[s][jt][:, 0:65],
                                 p01[:, 0:cw], start=(jt == 0),
                                 stop=(jt == njt - 1))
                nc.tensor.matmul(pv1[0:65, AW - cw:AW], vts[s][jt][:, 65:130],
                                 p01[:, AW:AW + cw], start=(jt == 0),
                                 stop=(jt == njt - 1))

            for jt in range(njt):
                r = jt - ACT_PER * c
                cw = AW - 128 * r if r > 0 else AW
                ioff = c * AW + (AW - cw)
                s01 = ps.tile([128, 2 * AW], f32, tag="s", bufs=2)
                nc.tensor.matmul(s01[:, 0:cw],
                                 kts[s][0:64, jt * 128:(jt + 1) * 128],
                                 qts[s][0:64, ioff:ioff + cw],
                                 start=True, stop=True)
                nc.tensor.matmul(s01[:, AW:AW + cw],
                                 kts[s][64:128, jt * 128:(jt + 1) * 128],
                                 qts[s][64:128, ioff:ioff + cw],
                                 start=True, stop=True)
                p01 = sb_l.tile([128, 2 * AW], bf16, tag="pt", bufs=3)
                s3 = s01.rearrange("p (h m) -> p h m", h=2)
                p3 = p01.rearrange("p (h m) -> p h m", h=2)
                nc.scalar.activation(p3[:, :, 0:cw], s3[:, :, 0:cw], Exp)
                if r >= 0:
                    for off in (0, AW):
                        nc.gpsimd.affine_select(
                            out=p01[:, off:off + 128], in_=p01[:, off:off + 128],
                            compare_op=mybir.AluOpType.is_ge, fill=0.0,
                            base=0, pattern=[[1, 128]], channel_multiplier=-1)
                stash[jt] = (p01, cw)
                pop_filler()
                if jt > 1:
                    emit_pv(jt - 2)
            if njt > 1:
                pop_filler()
                emit_pv(njt - 2)
            pop_filler()
            emit_pv(njt - 1)
            # normalize: oT[d, i] = pv[d, i] / denom[i]; denom at psum row 64.
            # Reciprocals are emitted now (DVE runs them while the PE moves on);
            # the PE broadcast matmul + remaining DVE ops are queued and popped
            # as fillers early in the next chunk, so the PE never waits on the
            # reciprocal and the pv psum buffers drain with slack to spare.
            cwin = slice(c * AW, (c + 1) * AW)
            for par, pvx in ((0, pv0), (1, pv1)):
                rr = sb_l.tile([65, AW], f32r, tag="rr", bufs=4, name="rr")
                with nc.allow_low_precision("f32r reciprocal of softmax denom"):
                    nc.vector.reciprocal(rr[64:65, :], pvx[64:65, :])

                def norm_tail(par=par, pvx=pvx, rr=rr):
                    bp = ps.tile([64, AW], f32, tag="acc", bufs=2, name="bp")
                    nc.tensor.matmul(bp[0:64, :], ones_t[64:65, 0:64],
                                     rr[64:65, :], start=True, stop=True)
                    # DVE may read only one PSUM operand; stage via ACT, which
                    # is idle at chunk boundaries where these units pop
                    bb = sb_l.tile([64, AW], f32r, tag="bb", bufs=4, name="bb")
                    nc.scalar.activation(bb[:], bp[0:64, :],
                                         mybir.ActivationFunctionType.Copy)
                    if par == 0:
                        nc.vector.tensor_mul(oTs[s][0:64, cwin], pvx[0:64, :],
                                             bb[:])
                    else:
                        tm = sb_l.tile([64, AW], f32r, tag="tm", bufs=2,
                                       name="tm")
                        nc.vector.tensor_mul(tm[:], pvx[0:64, :], bb[:])
                        nc.gpsimd.dma_start(out=oTs[s][64:128, cwin], in_=tm[:])
                if c == NAC - 1:
                    norm_tail()
                else:
                    norm_q.append((100, norm_tail))

        # ---------- emission ----------
        dma_x_chunk(0)
        for s in range(4):
            dma_wqk(s)
        dma_wv()
        nc.gpsimd.dma_start(out=gv_sb[:], in_=gv_d[:])
        nc.gpsimd.dma_start(out=ident_t[:], in_=id_d[:])
        dma_wp()
        # preload the Exp activation table off the critical path
        nc.vector.memset(scr[:], 0.0)
        nc.scalar.activation(scrb[:], scr[:], Exp)

        def p_units(tck):
            us = []
            for s in range(4):
                us += units_q(s, tck)
                us += units_k(s, tck)
            for tt in range(4 * tck, 4 * tck + 4):
                us += units_v(tt)
            return us

        # chunk 0 projections run straight (nothing to overlap them with)
        dma_x_chunk(1)
        for _, u in p_units(0):
            u()

        CPX = 512 // AW  # attention chunks per x-chunk
        fill_q.extend([(co, fn, 1) for co, fn in p_units(1)])
        for c in range(NAC):
            xc = c // CPX            # x-chunk this attention chunk lives in
            if c % CPX == 0 and xc + 2 <= 3:
                # 2-window lookahead: x tiles are double buffered, so P(xc+2)
                # may start as soon as P(xc) consumers are done
                dma_x_chunk(xc + 2)
                fill_q.extend([(co, fn, xc + 2) for co, fn in p_units(xc + 2)])
            for s in range(4):
                emit_attn(s, c)
            # P(xc+1) must complete before attention enters x-chunk xc+1
            if c % CPX == CPX - 1:
                flush_fillers(upto_win=xc + 1)
            if c % CPX == CPX - 1 and c // CPX == 0:
                # window 2 also runs filler-lean: release the first chunk's
                # projections (ready since the end of window 0) there
                for tt in range(0, 4):
                    for ec in range(2):
                        proj_q.extend(units_proj(tt, ec))
            if c % CPX == CPX - 1 and c // CPX == 2:
                for tt in range(4, 12):
                    for ec in range(2):
                        proj_q.extend(units_proj(tt, ec))
        flush_fillers(proj=True)
        for tt in range(12, 16):
            for ec in range(2):
                for _, u in units_proj(tt, ec):
                    u()

    nc.finalize()
    return nc


def _get_nc():
    global _NC_CACHE
    if _NC_CACHE is None:
        _NC_CACHE = _build_nc()
    return _NC_CACHE


def _slopes():
    start = 2.0 ** (-(2.0 ** (-(math.log2(H) - 3.0))))
    return np.array([start * start ** i for i in range(H)], dtype=np.float64)


def _kmajor(wT):
    # [E, 512] -> [128, slot(4) x ktile(8) x 128] so each slot's weight DMA is
    # one contiguous [128, 1024] transfer
    out = np.empty((128, 4096), dtype=np.float64)
    for s in range(4):
        blk = wT[:, s * 128:(s + 1) * 128].reshape(8, 128, 128)
        out[:, s * 1024:(s + 1) * 1024] = blk.transpose(1, 0, 2).reshape(128, 1024)
    return np.ascontiguousarray(out).astype(BF16)


_IDENT = np.eye(128, dtype=np.float32).astype(BF16)


def _host_prep(x, Wq, Aq, Bq, Wk, Ak, Bk, Wv, Av, Bv, Wp):
    f8 = np.float64
    weff = {}
    for nm, W, A, B in (("q", Wq, Aq, Bq), ("k", Wk, Ak, Bk), ("v", Wv, Av, Bv)):
        weff[nm] = (W.astype(f8) + 2.0 * (A.astype(f8) @ B.astype(f8)))
    weff["q"] = weff["q"] / math.sqrt(DH)
    slopes = _slopes()

    in_maps = []
    for b in range(4):
        xTb = np.ascontiguousarray(x[b].T).astype(BF16)
        for pairs in (SET_A, SET_B):
            perm = [h * 64 + k for pp in pairs for h in pp for k in range(64)]
            gv = np.zeros((128, 128), dtype=np.float32)
            pj = np.arange(128, dtype=f8)
            for s, (h0, h1) in enumerate(pairs):
                for tt in range(SLOT_JN[s]):
                    j = tt * 128 + pj
                    g0 = 2 * (SLOT_BASE[s] + tt)
                    for hi, h in enumerate((h0, h1)):
                        col = np.exp(-(slopes[h] * j + CB))
                        gv[:, g0 + hi] = col.astype(np.float32)
            in_maps.append({
                "xT": xTb,
                "wqT": _kmajor(weff["q"][perm].T),
                "wkT": _kmajor(weff["k"][perm].T),
                "wvT": np.ascontiguousarray(weff["v"][perm].T).astype(BF16),
                "wpT": np.ascontiguousarray(Wp[:, perm].T).astype(np.float32),
                "gvt": gv,
                "identb": _IDENT,
            })
    return in_maps


def run(inputs, trace=False):
    nc = _get_nc()
    inputs = {k: np.asarray(v, dtype=np.float32) for k, v in inputs.items()}
    in_maps = _host_prep(**inputs)
    res = run_bass_kernel_spmd(nc, in_maps, list(range(8)), trace=trace)
    outs = [np.asarray(res.results[i]["out"]) for i in range(8)]
    full = np.stack([outs[2 * b] + outs[2 * b + 1] for b in range(4)])
    return full.astype(np.float32), res


def kernel(**inputs):
    full, _ = run(inputs, trace=False)
    return full
